# revision 1
# baseline (speedup 1.0000x reference)
"""Trainium2 Bass kernel for nn_CrossModalAttention (B=16384, GNN=512, TR=768, F=1024).

Math (seq_len==1 degenerate attention, see reference):
    gp = g @ Wg.T + bg                       [B, F]
    tp = t @ Wt.T + bt                       [B, F]
    ga = (tp @ Wv.T + bv) @ Wo.T + bo        (attention(g, t, t))
    ta = (gp @ Wv.T + bv) @ Wo.T + bo
    h  = gelu([ga, ta] @ W1.T + b1)
    out = h @ W2.T + b2 + gp + tp

The attention block is affine, so it folds into W1 on the host:
    M1 = W1[:, :F] @ Wo @ Wv   (multiplies tp)
    M2 = W1[:, F:] @ Wo @ Wv   (multiplies gp)
    c  = (W1[:, :F] + W1[:, F:]) @ (Wo @ bv + bo) + b1
    h  = gelu(M1 @ tp.T + M2 @ gp.T + c)     (transposed layout)

Device kernel works in transposed layout [feature, batch] so the matmul
contraction dim always lands on SBUF partitions; host transposes in/out.
Data parallel over 8 cores: each core owns 2048 batch rows.
"""

import sys

import numpy as np

for _p in ("/opt/trn_rl_repo", "/root/.axon_site/_ro/trn_rl_repo"):
    if _p not in sys.path:
        sys.path.append(_p)

import ml_dtypes

import concourse.bass as bass
import concourse.mybir as mybir
import concourse.tile as tile
from concourse.bass import ts
from concourse.bass_utils import run_bass_kernel_spmd

B = 16384
GNN = 512
TR = 768
F = 1024
N_CORES = 8
B_LOC = B // N_CORES  # 2048
P = 128

# Stage dtypes: AB = the gp/tp projections (dominant output terms),
# CD = the folded-attention/fusion branch (small contribution to output).
# "bf16x2" = hi/lo bf16 split of inputs+weights, 3 matmuls per K-tile
# (drops only the lo*lo term): ~1e-5 rel err at 3x bf16 cost.
AB_DT = "f32r"  # "f32r" | "bf16" | "f32" | "bf16x2"
CD_DT = "bf16"  # "bf16" | "f32r" | "f32"
NB = 512  # batch-column block per step
PSUM_BUFS = 8
IO_BUFS = 1
AF = mybir.ActivationFunctionType


def _np_dt(sdt):
    return ml_dtypes.bfloat16 if sdt == "bf16" else np.float32


def _mb_dt(sdt):
    return {
        "bf16": mybir.dt.bfloat16,
        "f32r": mybir.dt.float32r,
        "f32": mybir.dt.float32,
    }[sdt]


def _mm_cast(ap, sdt):
    """Bitcast a float32 AP to float32r for reduced-precision full-rate matmul."""
    if sdt == "f32r":
        return ap.bitcast(mybir.dt.float32r)
    return ap


_DMA_OPCODES = ("DMACopy", "DMATranspose", "EventSemaphore", "TriggeredCopy")


def _legalize_waits(bir: dict) -> dict:
    """Walrus on this stack accepts only ONE sync-wait per engine instruction
    ("Too many sync wait commands"). Hoist extra waits onto standalone
    EventSemaphore ops (what nc.<engine>.wait_ge emits) on the same engine."""
    ctr = 0

    def hoist(out, inst, w):
        nonlocal ctr
        ctr += 1
        out.append(
            {
                "debug": inst.get("debug", 0),
                "engine": inst["engine"],
                "ins": [],
                "outs": [],
                "name": f"I-lgw-{ctr}",
                "opcode": "EventSemaphore",
                "sync_info": {"on_update": [], "on_wait": [w]},
            }
        )

    for fn in bir["functions"]:
        for blk in fn["blocks"]:
            out = []
            for inst in blk["instructions"]:
                si = inst.get("sync_info")
                waits = (si.get("on_wait") or []) if si else []
                op = inst.get("opcode")
                if op == "EventSemaphore":
                    pass
                elif op in ("DMACopy", "DMATranspose", "TriggeredCopy"):
                    # keep one wait (prefer a queue DMA* sem) on the descriptor,
                    # hoist the rest onto the issuing sequencer
                    if len(waits) > 1:
                        keep = [w for w in waits if w["ant_name"].startswith("DMA")]
                        drop = [w for w in waits if not w["ant_name"].startswith("DMA")]
                        if not keep:
                            keep = [waits[-1]]
                            drop = waits[:-1]
                        while len(keep) > 1:
                            drop.append(keep.pop(0))
                        for w in drop:
                            hoist(out, inst, w)
                        si["on_wait"] = keep
                elif len(waits) > 1:
                    for w in waits[:-1]:
                        hoist(out, inst, w)
                    si["on_wait"] = waits[-1:]
                out.append(inst)
            blk["instructions"] = out
    return bir


def _attach_wait_legalizer(nc):
    import json as _json

    orig_fn = nc.to_json_bytes

    def _patched():
        bir = _json.loads(orig_fn())
        _legalize_waits(bir)
        return _json.dumps(bir).encode()

    nc.to_json_bytes = _patched


def build_module(repeat=1):
    nc = bass.Bass()
    f32 = mybir.dt.float32
    # tensors consumed by an fp32r matmul must themselves be declared fp32r
    # end-to-end (walrus birverifier "not rounded to FP32r" check)
    ab_io = _mb_dt(AB_DT)
    cd_io = _mb_dt(CD_DT)

    gT = nc.dram_tensor("gT", [GNN, B_LOC], ab_io, kind="ExternalInput")
    tT = nc.dram_tensor("tT", [TR, B_LOC], ab_io, kind="ExternalInput")
    wgT = nc.dram_tensor("wgT", [GNN, F], ab_io, kind="ExternalInput")
    wtT = nc.dram_tensor("wtT", [TR, F], ab_io, kind="ExternalInput")
    mcT = nc.dram_tensor("mcT", [2 * F, F], cd_io, kind="ExternalInput")
    w2T = nc.dram_tensor("w2T", [F, F], cd_io, kind="ExternalInput")
    bg = nc.dram_tensor("bg", [F], f32, kind="ExternalInput")
    bt = nc.dram_tensor("bt", [F], f32, kind="ExternalInput")
    cv = nc.dram_tensor("cv", [F], f32, kind="ExternalInput")
    b2 = nc.dram_tensor("b2", [F], f32, kind="ExternalInput")
    outT = nc.dram_tensor("outT", [F, B_LOC], f32, kind="ExternalOutput")

    KG = GNN // P  # 4
    KT = TR // P  # 6
    KF = F // P  # 8
    NBLK = B_LOC // NB

    g_ap = gT[:].rearrange("(k p) b -> p k b", p=P)
    t_ap = tT[:].rearrange("(k p) b -> p k b", p=P)
    out_ap = outT[:].rearrange("(k p) b -> p k b", p=P)

    with tile.TileContext(nc) as tc:
        with (
            tc.tile_pool(name="const", bufs=1) as const,
            tc.tile_pool(name="io", bufs=IO_BUFS) as io,
            tc.tile_pool(name="act", bufs=1) as act,
            tc.tile_pool(name="psum", bufs=PSUM_BUFS, space="PSUM") as psum,
        ):
            wg = const.tile([P, KG, F], _mb_dt(AB_DT))
            nc.sync.dma_start(out=wg, in_=wgT[:].rearrange("(k p) f -> p k f", p=P))
            wt = const.tile([P, KT, F], _mb_dt(AB_DT))
            nc.sync.dma_start(out=wt, in_=wtT[:].rearrange("(k p) f -> p k f", p=P))
            bg_t = const.tile([P, KF], f32)
            nc.sync.dma_start(out=bg_t, in_=bg[:].rearrange("(k p) -> p k", p=P))
            bt_t = const.tile([P, KF], f32)
            nc.sync.dma_start(out=bt_t, in_=bt[:].rearrange("(k p) -> p k", p=P))
            cv_t = const.tile([P, KF], f32)
            nc.sync.dma_start(out=cv_t, in_=cv[:].rearrange("(k p) -> p k", p=P))
            b2_t = const.tile([P, KF], f32)
            nc.sync.dma_start(out=b2_t, in_=b2[:].rearrange("(k p) -> p k", p=P))
            mc = const.tile([P, 2 * KF, F], _mb_dt(CD_DT))
            nc.sync.dma_start(out=mc, in_=mcT[:].rearrange("(k p) f -> p k f", p=P))
            w2 = const.tile([P, KF, F], _mb_dt(CD_DT))
            nc.sync.dma_start(out=w2, in_=w2T[:].rearrange("(k p) f -> p k f", p=P))

            for blk in [b for _ in range(repeat) for b in range(NBLK)]:
                bs = slice(blk * NB, (blk + 1) * NB)
                g_in = io.tile([P, KG, NB], wg.dtype, tag="g_in")
                nc.sync.dma_start(out=g_in, in_=g_ap[:, :, bs])
                t_in = io.tile([P, KT, NB], wt.dtype, tag="t_in")
                nc.sync.dma_start(out=t_in, in_=t_ap[:, :, bs])

                act_dt = mybir.dt.float32r if CD_DT == "f32r" else f32
                gp = act.tile([P, KF, NB], act_dt, tag="gp")
                tp = act.tile([P, KF, NB], act_dt, tag="tp")
                if CD_DT == "bf16":
                    gpb = act.tile([P, KF, NB], mybir.dt.bfloat16, tag="gpb")
                    tpb = act.tile([P, KF, NB], mybir.dt.bfloat16, tag="tpb")

                # A: gp = Wg @ g (+bg);  B: tp = Wt @ t (+bt)
                for w_t, x_in, y, yb, b_t, kk in (
                    (wg, g_in, gp, "gpb", bg_t, KG),
                    (wt, t_in, tp, "tpb", bt_t, KT),
                ):
                    for j in range(KF):
                        ps = psum.tile([P, NB], f32, tag="ps")
                        for k in range(kk):
                            nc.tensor.matmul(
                                ps,
                                _mm_cast(w_t[:, k, ts(j, P)], AB_DT),
                                _mm_cast(x_in[:, k, :], AB_DT),
                                start=(k == 0),
                                stop=(k == kk - 1),
                            )
                        nc.scalar.activation(y[:, j, :], ps, AF.Identity, bias=b_t[:, j : j + 1])
                        if CD_DT == "bf16":
                            dst = gpb if yb == "gpb" else tpb
                            nc.vector.tensor_copy(dst[:, j, :], y[:, j, :])

                # C: h = gelu(M2 @ gp + M1 @ tp + c)   (gp half first: ready earlier)
                rhs_g = gpb if CD_DT == "bf16" else gp
                rhs_t = tpb if CD_DT == "bf16" else tp
                h = act.tile([P, KF, NB], mc.dtype, tag="h")
                for j in range(KF):
                    ps = psum.tile([P, NB], f32, tag="ps")
                    for k in range(KF):
                        nc.tensor.matmul(
                            ps,
                            _mm_cast(mc[:, KF + k, ts(j, P)], CD_DT),
                            _mm_cast(rhs_g[:, k, :], CD_DT),
                            start=(k == 0),
                            stop=False,
                        )
                    for k in range(KF):
                        nc.tensor.matmul(
                            ps,
                            _mm_cast(mc[:, k, ts(j, P)], CD_DT),
                            _mm_cast(rhs_t[:, k, :], CD_DT),
                            start=False,
                            stop=(k == KF - 1),
                        )
                    nc.scalar.activation(h[:, j, :], ps, AF.Gelu, bias=cv_t[:, j : j + 1])

                # D: out = W2 @ h + b2 + gp + tp
                # epilogue all on DVE so the out DMA has a single-engine dep
                out_t = io.tile([P, KF, NB], f32, tag="out_t")
                for j in range(KF):
                    ps = psum.tile([P, NB], f32, tag="ps")
                    for k in range(KF):
                        nc.tensor.matmul(
                            ps,
                            _mm_cast(w2[:, k, ts(j, P)], CD_DT),
                            _mm_cast(h[:, k, :], CD_DT),
                            start=(k == 0),
                            stop=(k == KF - 1),
                        )
                    nc.vector.tensor_scalar_add(out_t[:, j, :], ps, b2_t[:, j : j + 1])
                    nc.vector.tensor_add(out_t[:, j, :], out_t[:, j, :], gp[:, j, :])
                    nc.vector.tensor_add(out_t[:, j, :], out_t[:, j, :], tp[:, j, :])
                nc.sync.dma_start(out=out_ap[:, :, bs], in_=out_t)

    _attach_wait_legalizer(nc)
    return nc


def prepare_inputs(gnn_features, transformer_features, Wg, bg, Wt, bt, Wv, bv, Wo, bo, W1, b1, W2, b2):
    """Host-side: fold the affine attention block into W1, transpose everything."""
    f64 = np.float64
    A = Wo.astype(f64) @ Wv.astype(f64)
    W1a = W1[:, :F].astype(f64)
    W1b = W1[:, F:].astype(f64)
    M1 = W1a @ A
    M2 = W1b @ A
    c = (W1a + W1b) @ (Wo.astype(f64) @ bv.astype(f64) + bo.astype(f64)) + b1.astype(f64)

    ab_np = _np_dt(AB_DT)
    cd_np = _np_dt(CD_DT)
    wgT = np.ascontiguousarray(Wg.T).astype(ab_np)
    wtT = np.ascontiguousarray(Wt.T).astype(ab_np)
    mcT = np.ascontiguousarray(np.concatenate([M1.T, M2.T], axis=0).astype(np.float32)).astype(cd_np)
    w2T = np.ascontiguousarray(W2.T).astype(cd_np)

    shared = {
        "wgT": wgT,
        "wtT": wtT,
        "mcT": mcT,
        "w2T": w2T,
        "bg": np.asarray(bg, np.float32),
        "bt": np.asarray(bt, np.float32),
        "cv": c.astype(np.float32),
        "b2": np.asarray(b2, np.float32),
    }
    in_maps = []
    for i in range(N_CORES):
        rows = slice(i * B_LOC, (i + 1) * B_LOC)
        in_maps.append(
            {
                "gT": np.ascontiguousarray(gnn_features[rows].T).astype(ab_np),
                "tT": np.ascontiguousarray(transformer_features[rows].T).astype(ab_np),
                **shared,
            }
        )
    return in_maps


def run(inputs, trace=False, **kw):
    nc = build_module()
    in_maps = prepare_inputs(**inputs)
    res = run_bass_kernel_spmd(nc, in_maps, core_ids=list(range(N_CORES)), trace=trace, **kw)
    out = np.concatenate([r["outT"].T for r in res.results], axis=0).astype(np.float32)
    return out, res


def kernel(**inputs) -> np.ndarray:
    out, _ = run(inputs, trace=False)
    return out



# revision 2
# speedup vs baseline: 2.4665x; 2.4665x over previous
"""Trainium2 Bass kernel for nn_CrossModalAttention (B=16384, GNN=512, TR=768, F=1024).

Math (seq_len==1 degenerate attention, see reference):
    gp = g @ Wg.T + bg                       [B, F]
    tp = t @ Wt.T + bt                       [B, F]
    ga = (tp @ Wv.T + bv) @ Wo.T + bo        (attention(g, t, t))
    ta = (gp @ Wv.T + bv) @ Wo.T + bo
    h  = gelu([ga, ta] @ W1.T + b1)
    out = h @ W2.T + b2 + gp + tp

The whole affine attention+fusion prefix folds down to the raw inputs
on the host:
    P1 = W1[:, :F] @ Wo @ Wv @ Wt            [F, TR]   (multiplies t)
    P2 = W1[:, F:] @ Wo @ Wv @ Wg            [F, GNN]  (multiplies g)
    cv = (W1[:,:F]+W1[:,F:]) @ (Wo@bv+bo) + b1 + P-folded bias terms
    h  = gelu(P1 @ t.T + P2 @ g.T + cv)               [F, B] transposed
    out = W2 @ h + Wg @ g.T + Wt @ t.T + (bg+bt+b2)

Device kernel (transposed [feature, batch] layout, data parallel over 8
cores, 2048 batch rows each, 4 column blocks of 512):
  C phase: h = gelu(P12 @ [t;g])  -- fp8e4 DoubleRow matmuls (2 K-slabs
           per pass at 0.5 cycles/row), P12 pre-scaled by a power of two
           (absmax -> ~224) on host, descaled in the Gelu activation.
  D phase: one PSUM accumulation per output tile sums THREE matmul
           groups: W2@h (fp8 DoubleRow, scaled s2), Wg@g and Wt@t (bf16,
           weights pre-scaled by the same s2 so a single epilogue
           activation descales everything and adds the bias). gp/tp are
           never materialized.
fp8 tensors travel as uint8 DRAM/SBUF and are bitcast to float8e4 at
the matmul/activation, so the host<->device path never sees fp8 dtypes.
"""

import sys

import numpy as np

for _p in ("/opt/trn_rl_repo", "/root/.axon_site/_ro/trn_rl_repo"):
    if _p not in sys.path:
        sys.path.append(_p)

import ml_dtypes

import concourse.bass as bass
import concourse.mybir as mybir
import concourse.tile as tile
from concourse.bass import ts
from concourse.bass_utils import run_bass_kernel_spmd

B = 16384
GNN = 512
TR = 768
F = 1024
N_CORES = 8
B_LOC = B // N_CORES  # 2048
P = 128

KG = GNN // P  # 4
KT = TR // P  # 6
KC = KT + KG  # 10 contraction slabs for the C (fused) stage
KF = F // P  # 8

NB = 512  # batch-column block per step
NBLK = B_LOC // NB
PSUM_BUFS = 8
IO_BUFS = 2
AF = mybir.ActivationFunctionType
DR = mybir.MatmulPerfMode.DoubleRow
FP8 = mybir.dt.float8e4

_DMA_OPCODES = ("DMACopy", "DMATranspose", "EventSemaphore", "TriggeredCopy")


def _legalize_waits(bir: dict) -> dict:
    """Walrus on this stack accepts only ONE sync-wait per engine instruction
    ("Too many sync wait commands"). Hoist extra waits onto standalone
    EventSemaphore ops (what nc.<engine>.wait_ge emits) on the same engine."""
    ctr = 0

    def hoist(out, inst, w):
        nonlocal ctr
        ctr += 1
        out.append(
            {
                "debug": inst.get("debug", 0),
                "engine": inst["engine"],
                "ins": [],
                "outs": [],
                "name": f"I-lgw-{ctr}",
                "opcode": "EventSemaphore",
                "sync_info": {"on_update": [], "on_wait": [w]},
            }
        )

    for fn in bir["functions"]:
        for blk in fn["blocks"]:
            out = []
            for inst in blk["instructions"]:
                si = inst.get("sync_info")
                waits = (si.get("on_wait") or []) if si else []
                op = inst.get("opcode")
                if op == "EventSemaphore":
                    pass
                elif op in ("DMACopy", "DMATranspose", "TriggeredCopy"):
                    # keep one wait (prefer a queue DMA* sem) on the descriptor,
                    # hoist the rest onto the issuing sequencer
                    if len(waits) > 1:
                        keep = [w for w in waits if w["ant_name"].startswith("DMA")]
                        drop = [w for w in waits if not w["ant_name"].startswith("DMA")]
                        if not keep:
                            keep = [waits[-1]]
                            drop = waits[:-1]
                        while len(keep) > 1:
                            drop.append(keep.pop(0))
                        for w in drop:
                            hoist(out, inst, w)
                        si["on_wait"] = keep
                elif len(waits) > 1:
                    for w in waits[:-1]:
                        hoist(out, inst, w)
                    si["on_wait"] = waits[-1:]
                out.append(inst)
            blk["instructions"] = out
    return bir


def _attach_wait_legalizer(nc):
    import json as _json

    orig_fn = nc.to_json_bytes

    def _patched():
        bir = _json.loads(orig_fn())
        _legalize_waits(bir)
        return _json.dumps(bir).encode()

    nc.to_json_bytes = _patched


def build_module(sp_inv=1.0 / 4096, s2_inv=1.0 / 2048, repeat=1):
    nc = bass.Bass()
    f32 = mybir.dt.float32
    bf16 = mybir.dt.bfloat16
    u8 = mybir.dt.uint8

    gT = nc.dram_tensor("gT", [GNN, B_LOC], bf16, kind="ExternalInput")
    tT = nc.dram_tensor("tT", [TR, B_LOC], bf16, kind="ExternalInput")
    g8T = nc.dram_tensor("g8T", [GNN, B_LOC], u8, kind="ExternalInput")
    t8T = nc.dram_tensor("t8T", [TR, B_LOC], u8, kind="ExternalInput")
    p12T = nc.dram_tensor("p12T", [KC * P, F], u8, kind="ExternalInput")
    wgT = nc.dram_tensor("wgT", [GNN, F], bf16, kind="ExternalInput")
    wtT = nc.dram_tensor("wtT", [TR, F], bf16, kind="ExternalInput")
    w2T = nc.dram_tensor("w2T", [F, F], u8, kind="ExternalInput")
    cv = nc.dram_tensor("cv", [F], f32, kind="ExternalInput")
    bsum = nc.dram_tensor("bsum", [F], f32, kind="ExternalInput")
    outT = nc.dram_tensor("outT", [F, B_LOC], f32, kind="ExternalOutput")

    g_ap = gT[:].rearrange("(k p) b -> p k b", p=P)
    t_ap = tT[:].rearrange("(k p) b -> p k b", p=P)
    g8_ap = g8T[:].rearrange("(k p) b -> p k b", p=P)
    t8_ap = t8T[:].rearrange("(k p) b -> p k b", p=P)
    p12_ap = p12T[:].rearrange("(k p) f -> p k f", p=P)
    out_ap = outT[:].rearrange("(k p) b -> p k b", p=P)

    with tile.TileContext(nc) as tc:
        with (
            tc.tile_pool(name="const", bufs=1) as const,
            tc.tile_pool(name="io", bufs=IO_BUFS) as io,
            tc.tile_pool(name="hbuf", bufs=2) as hbuf,
            tc.tile_pool(name="psum", bufs=PSUM_BUFS, space="PSUM") as psum,
        ):
            # ---- constants; DMA order puts the C-stage needs first so the
            # tensor engine starts ~4us in, streaming the rest under compute.
            p12 = const.tile([P, KC, F], u8)
            cv_t = const.tile([P, KF], f32)
            wg = const.tile([P, KG, F], bf16)
            wt = const.tile([P, KT, F], bf16)
            w2 = const.tile([P, KF, F], u8)
            bs_t = const.tile([P, KF], f32)

            # p12 split per output tile: C_0 can start after 1/8 of it
            for j in range(KF):
                nc.sync.dma_start(out=p12[:, :, ts(j, P)], in_=p12_ap[:, :, ts(j, P)])
            nc.sync.dma_start(out=cv_t, in_=cv[:].rearrange("(k p) -> p k", p=P))

            for blk in [b for _ in range(repeat) for b in range(NBLK)]:
                bs = slice(blk * NB, (blk + 1) * NB)
                g8 = io.tile([P, KG, NB], u8, tag="g8")
                nc.sync.dma_start(out=g8, in_=g8_ap[:, :, bs])
                t8 = io.tile([P, KT, NB], u8, tag="t8")
                nc.sync.dma_start(out=t8, in_=t8_ap[:, :, bs])
                g_in = io.tile([P, KG, NB], bf16, tag="g_in")
                t_in = io.tile([P, KT, NB], bf16, tag="t_in")
                if blk == 0:
                    # D-phase weights stream while the first C phase runs
                    nc.sync.dma_start(out=wg, in_=wgT[:].rearrange("(k p) f -> p k f", p=P))
                    nc.sync.dma_start(out=g_in, in_=g_ap[:, :, bs])
                    nc.sync.dma_start(out=wt, in_=wtT[:].rearrange("(k p) f -> p k f", p=P))
                    nc.sync.dma_start(out=t_in, in_=t_ap[:, :, bs])
                    nc.sync.dma_start(out=w2, in_=w2T[:].rearrange("(k p) f -> p k f", p=P))
                    nc.sync.dma_start(out=bs_t, in_=bsum[:].rearrange("(k p) -> p k", p=P))
                else:
                    nc.sync.dma_start(out=g_in, in_=g_ap[:, :, bs])
                    nc.sync.dma_start(out=t_in, in_=t_ap[:, :, bs])

                h8 = hbuf.tile([P, KF, NB], u8, tag="h8")
                out_t = io.tile([P, KF, NB], f32, tag="out_t")

                # C: h = gelu((P1@t + P2@g) * sp_inv + cv), fp8 DoubleRow
                for j in range(KF):
                    ps = psum.tile([P, NB], f32, tag="ps")
                    for kk in range(0, KT, 2):
                        nc.tensor.matmul(
                            ps,
                            p12[:, kk : kk + 2, ts(j, P)].bitcast(FP8),
                            t8[:, kk : kk + 2, :].bitcast(FP8),
                            start=(kk == 0),
                            stop=False,
                            perf_mode=DR,
                        )
                    for kk in range(0, KG, 2):
                        nc.tensor.matmul(
                            ps,
                            p12[:, KT + kk : KT + kk + 2, ts(j, P)].bitcast(FP8),
                            g8[:, kk : kk + 2, :].bitcast(FP8),
                            start=False,
                            stop=(kk + 2 >= KG),
                            perf_mode=DR,
                        )
                    nc.scalar.activation(
                        h8[:, j, :].bitcast(FP8), ps, AF.Gelu, bias=cv_t[:, j : j + 1], scale=sp_inv
                    )

                # D: out = (W2@h + Wg@g + Wt@t) * s2_inv + bsum
                # one PSUM group per tile; bf16 input-proj matmuls first (no h
                # dependency), fp8 DoubleRow W2@h last.
                for j in range(KF):
                    ps = psum.tile([P, NB], f32, tag="ps")
                    for k in range(KG):
                        nc.tensor.matmul(ps, wg[:, k, ts(j, P)], g_in[:, k, :], start=(k == 0), stop=False)
                    for k in range(KT):
                        nc.tensor.matmul(ps, wt[:, k, ts(j, P)], t_in[:, k, :], start=False, stop=False)
                    for kk in range(0, KF, 2):
                        nc.tensor.matmul(
                            ps,
                            w2[:, kk : kk + 2, ts(j, P)].bitcast(FP8),
                            h8[:, kk : kk + 2, :].bitcast(FP8),
                            start=False,
                            stop=(kk + 2 >= KF),
                            perf_mode=DR,
                        )
                    nc.scalar.activation(
                        out_t[:, j, :], ps, AF.Identity, bias=bs_t[:, j : j + 1], scale=s2_inv
                    )
                    if j % 2 == 1:
                        # out DMA on the Activation HWDGE queue: keeps the SP
                        # input queue free of head-of-line blocking
                        nc.scalar.dma_start(
                            out=out_ap[:, j - 1 : j + 1, bs], in_=out_t[:, j - 1 : j + 1, :]
                        )

    _attach_wait_legalizer(nc)
    return nc


def _pow2_scale(x, target=224.0):
    m = float(np.abs(x).max())
    if m == 0.0 or not np.isfinite(m):
        return 1.0
    return float(2.0 ** np.floor(np.log2(target / m)))


def prepare_inputs(gnn_features, transformer_features, Wg, bg, Wt, bt, Wv, bv, Wo, bo, W1, b1, W2, b2):
    """Host-side: fold attention+fusion prefix down to the raw inputs,
    quantize (fp8e4 as uint8 / bf16), transpose everything."""
    f64 = np.float64
    fp8 = ml_dtypes.float8_e4m3
    bf = ml_dtypes.bfloat16

    A = Wo.astype(f64) @ Wv.astype(f64)
    W1a = W1[:, :F].astype(f64)
    W1b = W1[:, F:].astype(f64)
    M1 = W1a @ A
    M2 = W1b @ A
    c = (W1a + W1b) @ (Wo.astype(f64) @ bv.astype(f64) + bo.astype(f64)) + b1.astype(f64)
    P1 = M1 @ Wt.astype(f64)  # [F, TR]
    P2 = M2 @ Wg.astype(f64)  # [F, GNN]
    cvec = c + M1 @ bt.astype(f64) + M2 @ bg.astype(f64)

    p12 = np.concatenate([P1.T, P2.T], axis=0)  # [TR+GNN, F] rows=contraction
    sp = _pow2_scale(p12)
    s2 = _pow2_scale(W2)

    p12T = np.ascontiguousarray(p12 * sp).astype(fp8).view(np.uint8)
    w2T = np.ascontiguousarray(W2.T.astype(f64) * s2).astype(fp8).view(np.uint8)
    wgT = np.ascontiguousarray(Wg.T.astype(f64) * s2).astype(bf)
    wtT = np.ascontiguousarray(Wt.T.astype(f64) * s2).astype(bf)
    bsum = (bg.astype(f64) + bt.astype(f64) + b2.astype(f64)).astype(np.float32)

    shared = {
        "p12T": p12T,
        "wgT": wgT,
        "wtT": wtT,
        "w2T": w2T,
        "cv": cvec.astype(np.float32),
        "bsum": bsum,
    }
    in_maps = []
    for i in range(N_CORES):
        rows = slice(i * B_LOC, (i + 1) * B_LOC)
        gTc = np.ascontiguousarray(gnn_features[rows].T)
        tTc = np.ascontiguousarray(transformer_features[rows].T)
        in_maps.append(
            {
                "gT": gTc.astype(bf),
                "tT": tTc.astype(bf),
                "g8T": gTc.astype(fp8).view(np.uint8),
                "t8T": tTc.astype(fp8).view(np.uint8),
                **shared,
            }
        )
    return in_maps, 1.0 / sp, 1.0 / s2


def run(inputs, trace=False, **kw):
    in_maps, sp_inv, s2_inv = prepare_inputs(**inputs)
    nc = build_module(sp_inv=sp_inv, s2_inv=s2_inv)
    res = run_bass_kernel_spmd(nc, in_maps, core_ids=list(range(N_CORES)), trace=trace, **kw)
    out = np.concatenate([r["outT"].T for r in res.results], axis=0).astype(np.float32)
    return out, res


def kernel(**inputs) -> np.ndarray:
    out, _ = run(inputs, trace=False)
    return out


# revision 4
# speedup vs baseline: 2.6901x; 1.0907x over previous
"""Trainium2 Bass kernel for nn_CrossModalAttention (B=16384, GNN=512, TR=768, F=1024).

Math (seq_len==1 degenerate attention, see reference):
    gp = g @ Wg.T + bg                       [B, F]
    tp = t @ Wt.T + bt                       [B, F]
    ga = (tp @ Wv.T + bv) @ Wo.T + bo        (attention(g, t, t))
    ta = (gp @ Wv.T + bv) @ Wo.T + bo
    h  = gelu([ga, ta] @ W1.T + b1)
    out = h @ W2.T + b2 + gp + tp

The whole affine attention+fusion prefix folds down to the raw inputs
on the host:
    P1 = W1[:, :F] @ Wo @ Wv @ Wt            [F, TR]   (multiplies t)
    P2 = W1[:, F:] @ Wo @ Wv @ Wg            [F, GNN]  (multiplies g)
    cv = (W1[:,:F]+W1[:,F:]) @ (Wo@bv+bo) + b1 + P-folded bias terms
    h  = gelu(P1 @ t.T + P2 @ g.T + cv)               [F, B] transposed
    out = W2 @ h + Wg @ g.T + Wt @ t.T + (bg+bt+b2)

Device kernel (transposed [feature, batch] layout, data parallel over 8
cores, 2048 batch rows each, 4 column blocks of 512):
  C phase: h = gelu(P12 @ [t;g])  -- fp8e4 DoubleRow matmuls (2 K-slabs
           per pass at 0.5 cycles/row), P12 pre-scaled by a power of two
           (absmax -> ~224) on host, descaled in the Gelu activation.
  D phase: one PSUM accumulation per output tile sums THREE matmul
           groups: W2@h (fp8 DoubleRow, scaled s2), Wg@g and Wt@t (bf16,
           weights pre-scaled by the same s2 so a single epilogue
           activation descales everything and adds the bias). gp/tp are
           never materialized.
fp8 tensors travel as uint8 DRAM/SBUF and are bitcast to float8e4 at
the matmul/activation, so the host<->device path never sees fp8 dtypes.
"""

import sys

import numpy as np

for _p in ("/opt/trn_rl_repo", "/root/.axon_site/_ro/trn_rl_repo"):
    if _p not in sys.path:
        sys.path.append(_p)

import ml_dtypes

import concourse.bass as bass
import concourse.mybir as mybir
import concourse.tile as tile
from concourse.bass import ts
from concourse.bass_utils import run_bass_kernel_spmd

B = 16384
GNN = 512
TR = 768
F = 1024
N_CORES = 8
B_LOC = B // N_CORES  # 2048
P = 128

KG = GNN // P  # 4
KT = TR // P  # 6
KC = KT + KG  # 10 contraction slabs for the C (fused) stage
KF = F // P  # 8

NB = 512  # batch-column block per step
NBLK = B_LOC // NB
PSUM_BUFS = 8
IO_BUFS = 2
AF = mybir.ActivationFunctionType
DR = mybir.MatmulPerfMode.DoubleRow
FP8 = mybir.dt.float8e4

_DMA_OPCODES = ("DMACopy", "DMATranspose", "EventSemaphore", "TriggeredCopy")


def _legalize_waits(bir: dict) -> dict:
    """Walrus on this stack accepts only ONE sync-wait per engine instruction
    ("Too many sync wait commands"). Hoist extra waits onto standalone
    EventSemaphore ops (what nc.<engine>.wait_ge emits) on the same engine."""
    ctr = 0

    def hoist(out, inst, w):
        nonlocal ctr
        ctr += 1
        out.append(
            {
                "debug": inst.get("debug", 0),
                "engine": inst["engine"],
                "ins": [],
                "outs": [],
                "name": f"I-lgw-{ctr}",
                "opcode": "EventSemaphore",
                "sync_info": {"on_update": [], "on_wait": [w]},
            }
        )

    for fn in bir["functions"]:
        for blk in fn["blocks"]:
            out = []
            for inst in blk["instructions"]:
                si = inst.get("sync_info")
                waits = (si.get("on_wait") or []) if si else []
                op = inst.get("opcode")
                if op == "EventSemaphore":
                    pass
                elif op in ("DMACopy", "DMATranspose", "TriggeredCopy"):
                    # keep one wait (prefer a queue DMA* sem) on the descriptor,
                    # hoist the rest onto the issuing sequencer
                    if len(waits) > 1:
                        keep = [w for w in waits if w["ant_name"].startswith("DMA")]
                        drop = [w for w in waits if not w["ant_name"].startswith("DMA")]
                        if not keep:
                            keep = [waits[-1]]
                            drop = waits[:-1]
                        while len(keep) > 1:
                            drop.append(keep.pop(0))
                        for w in drop:
                            hoist(out, inst, w)
                        si["on_wait"] = keep
                elif len(waits) > 1:
                    for w in waits[:-1]:
                        hoist(out, inst, w)
                    si["on_wait"] = waits[-1:]
                out.append(inst)
            blk["instructions"] = out
    return bir


def _attach_wait_legalizer(nc):
    import json as _json

    orig_fn = nc.to_json_bytes

    def _patched():
        bir = _json.loads(orig_fn())
        _legalize_waits(bir)
        return _json.dumps(bir).encode()

    nc.to_json_bytes = _patched


def build_module(sp_inv=1.0 / 4096, s2_inv=1.0 / 2048, repeat=1):
    nc = bass.Bass()
    f32 = mybir.dt.float32
    bf16 = mybir.dt.bfloat16
    u8 = mybir.dt.uint8

    gT = nc.dram_tensor("gT", [GNN, B_LOC], bf16, kind="ExternalInput")
    tT = nc.dram_tensor("tT", [TR, B_LOC], bf16, kind="ExternalInput")
    g8T = nc.dram_tensor("g8T", [GNN, B_LOC], u8, kind="ExternalInput")
    t8T = nc.dram_tensor("t8T", [TR, B_LOC], u8, kind="ExternalInput")
    p12T = nc.dram_tensor("p12T", [KC * P, F], u8, kind="ExternalInput")
    wgT = nc.dram_tensor("wgT", [GNN, F], bf16, kind="ExternalInput")
    wtT = nc.dram_tensor("wtT", [TR, F], bf16, kind="ExternalInput")
    w2T = nc.dram_tensor("w2T", [F, F], u8, kind="ExternalInput")
    cv = nc.dram_tensor("cv", [F], f32, kind="ExternalInput")
    bsum = nc.dram_tensor("bsum", [F], f32, kind="ExternalInput")
    outT = nc.dram_tensor("outT", [F, B_LOC], f32, kind="ExternalOutput")

    g_ap = gT[:].rearrange("(k p) b -> p k b", p=P)
    t_ap = tT[:].rearrange("(k p) b -> p k b", p=P)
    g8_ap = g8T[:].rearrange("(k p) b -> p k b", p=P)
    t8_ap = t8T[:].rearrange("(k p) b -> p k b", p=P)
    p12_ap = p12T[:].rearrange("(k p) f -> p k f", p=P)
    out_ap = outT[:].rearrange("(k p) b -> p k b", p=P)

    with tile.TileContext(nc) as tc:
        with (
            tc.tile_pool(name="const", bufs=1) as const,
            tc.tile_pool(name="io", bufs=IO_BUFS) as io,
            tc.tile_pool(name="hbuf", bufs=2) as hbuf,
            tc.tile_pool(name="psum", bufs=PSUM_BUFS, space="PSUM") as psum,
        ):
            # ---- constants; DMA order puts the C-stage needs first so the
            # tensor engine starts ~4us in, streaming the rest under compute.
            p12 = const.tile([P, KC, F], u8)
            cv_t = const.tile([P, KF], f32)
            wg = const.tile([P, KG, F], bf16)
            wt = const.tile([P, KT, F], bf16)
            w2 = const.tile([P, KF, F], u8)
            bs_t = const.tile([P, KF], f32)

            # p12 in two column halves: C_0 needs only the first half, the
            # second arrives while C_0..C_3 run
            half = (KF // 2) * P
            nc.sync.dma_start(out=p12[:, :, 0:half], in_=p12_ap[:, :, 0:half])

            for blk in [b for _ in range(repeat) for b in range(NBLK)]:
                bs = slice(blk * NB, (blk + 1) * NB)
                t8 = io.tile([P, KT, NB], u8, tag="t8")
                nc.sync.dma_start(out=t8, in_=t8_ap[:, :, bs])
                g8 = io.tile([P, KG, NB], u8, tag="g8")
                nc.sync.dma_start(out=g8, in_=g8_ap[:, :, bs])
                g_in = io.tile([P, KG, NB], bf16, tag="g_in")
                t_in = io.tile([P, KT, NB], bf16, tag="t_in")
                if blk == 0:
                    nc.sync.dma_start(out=cv_t, in_=cv[:].rearrange("(k p) -> p k", p=P))
                    nc.sync.dma_start(out=p12[:, :, half:F], in_=p12_ap[:, :, half:F])
                    # D-phase weights stream while the first C phase runs
                    nc.sync.dma_start(out=wg, in_=wgT[:].rearrange("(k p) f -> p k f", p=P))
                    nc.sync.dma_start(out=g_in, in_=g_ap[:, :, bs])
                    nc.sync.dma_start(out=wt, in_=wtT[:].rearrange("(k p) f -> p k f", p=P))
                    nc.sync.dma_start(out=t_in, in_=t_ap[:, :, bs])
                    nc.sync.dma_start(out=w2, in_=w2T[:].rearrange("(k p) f -> p k f", p=P))
                    nc.sync.dma_start(out=bs_t, in_=bsum[:].rearrange("(k p) -> p k", p=P))
                else:
                    nc.sync.dma_start(out=g_in, in_=g_ap[:, :, bs])
                    nc.sync.dma_start(out=t_in, in_=t_ap[:, :, bs])

                h8 = hbuf.tile([P, KF, NB], u8, tag="h8")
                out_t = io.tile([P, KF, NB], f32, tag="out_t")

                # C: h = gelu((P1@t + P2@g) * sp_inv + cv), fp8 DoubleRow
                for j in range(KF):
                    ps = psum.tile([P, NB], f32, tag="ps")
                    for kk in range(0, KT, 2):
                        nc.tensor.matmul(
                            ps,
                            p12[:, kk : kk + 2, ts(j, P)].bitcast(FP8),
                            t8[:, kk : kk + 2, :].bitcast(FP8),
                            start=(kk == 0),
                            stop=False,
                            perf_mode=DR,
                        )
                    for kk in range(0, KG, 2):
                        nc.tensor.matmul(
                            ps,
                            p12[:, KT + kk : KT + kk + 2, ts(j, P)].bitcast(FP8),
                            g8[:, kk : kk + 2, :].bitcast(FP8),
                            start=False,
                            stop=(kk + 2 >= KG),
                            perf_mode=DR,
                        )
                    nc.scalar.activation(
                        h8[:, j, :].bitcast(FP8), ps, AF.Gelu, bias=cv_t[:, j : j + 1], scale=sp_inv
                    )

                # D: out = (W2@h + Wg@g + Wt@t) * s2_inv + bsum
                # one PSUM group per tile; bf16 input-proj matmuls first (no h
                # dependency), fp8 DoubleRow W2@h last.
                for j in range(KF):
                    ps = psum.tile([P, NB], f32, tag="ps")
                    for k in range(KG):
                        nc.tensor.matmul(ps, wg[:, k, ts(j, P)], g_in[:, k, :], start=(k == 0), stop=False)
                    for k in range(KT):
                        nc.tensor.matmul(ps, wt[:, k, ts(j, P)], t_in[:, k, :], start=False, stop=False)
                    for kk in range(0, KF, 2):
                        nc.tensor.matmul(
                            ps,
                            w2[:, kk : kk + 2, ts(j, P)].bitcast(FP8),
                            h8[:, kk : kk + 2, :].bitcast(FP8),
                            start=False,
                            stop=(kk + 2 >= KF),
                            perf_mode=DR,
                        )
                    nc.scalar.activation(
                        out_t[:, j, :], ps, AF.Identity, bias=bs_t[:, j : j + 1], scale=s2_inv
                    )
                    # out DMA on the Activation HWDGE queue: keeps the SP
                    # input queue free of head-of-line blocking. Last block
                    # flushes per-j so the tail after the final matmul is one
                    # small chunk.
                    if blk == NBLK - 1:
                        nc.scalar.dma_start(out=out_ap[:, j : j + 1, bs], in_=out_t[:, j : j + 1, :])
                    elif j % 2 == 1:
                        nc.scalar.dma_start(
                            out=out_ap[:, j - 1 : j + 1, bs], in_=out_t[:, j - 1 : j + 1, :]
                        )

    _attach_wait_legalizer(nc)
    return nc


def _pow2_scale(x, target=224.0):
    m = float(np.abs(x).max())
    if m == 0.0 or not np.isfinite(m):
        return 1.0
    return float(2.0 ** np.floor(np.log2(target / m)))


def prepare_inputs(gnn_features, transformer_features, Wg, bg, Wt, bt, Wv, bv, Wo, bo, W1, b1, W2, b2):
    """Host-side: fold attention+fusion prefix down to the raw inputs,
    quantize (fp8e4 as uint8 / bf16), transpose everything."""
    f64 = np.float64
    fp8 = ml_dtypes.float8_e4m3
    bf = ml_dtypes.bfloat16

    A = Wo.astype(f64) @ Wv.astype(f64)
    W1a = W1[:, :F].astype(f64)
    W1b = W1[:, F:].astype(f64)
    M1 = W1a @ A
    M2 = W1b @ A
    c = (W1a + W1b) @ (Wo.astype(f64) @ bv.astype(f64) + bo.astype(f64)) + b1.astype(f64)
    P1 = M1 @ Wt.astype(f64)  # [F, TR]
    P2 = M2 @ Wg.astype(f64)  # [F, GNN]
    cvec = c + M1 @ bt.astype(f64) + M2 @ bg.astype(f64)

    p12 = np.concatenate([P1.T, P2.T], axis=0)  # [TR+GNN, F] rows=contraction
    sp = _pow2_scale(p12)
    s2 = _pow2_scale(W2)

    p12T = np.ascontiguousarray(p12 * sp).astype(fp8).view(np.uint8)
    w2T = np.ascontiguousarray(W2.T.astype(f64) * s2).astype(fp8).view(np.uint8)
    wgT = np.ascontiguousarray(Wg.T.astype(f64) * s2).astype(bf)
    wtT = np.ascontiguousarray(Wt.T.astype(f64) * s2).astype(bf)
    bsum = (bg.astype(f64) + bt.astype(f64) + b2.astype(f64)).astype(np.float32)

    shared = {
        "p12T": p12T,
        "wgT": wgT,
        "wtT": wtT,
        "w2T": w2T,
        "cv": cvec.astype(np.float32),
        "bsum": bsum,
    }
    in_maps = []
    for i in range(N_CORES):
        rows = slice(i * B_LOC, (i + 1) * B_LOC)
        gTc = np.ascontiguousarray(gnn_features[rows].T)
        tTc = np.ascontiguousarray(transformer_features[rows].T)
        in_maps.append(
            {
                "gT": gTc.astype(bf),
                "tT": tTc.astype(bf),
                "g8T": gTc.astype(fp8).view(np.uint8),
                "t8T": tTc.astype(fp8).view(np.uint8),
                **shared,
            }
        )
    return in_maps, 1.0 / sp, 1.0 / s2


def run(inputs, trace=False, **kw):
    in_maps, sp_inv, s2_inv = prepare_inputs(**inputs)
    nc = build_module(sp_inv=sp_inv, s2_inv=s2_inv)
    res = run_bass_kernel_spmd(nc, in_maps, core_ids=list(range(N_CORES)), trace=trace, **kw)
    out = np.concatenate([r["outT"].T for r in res.results], axis=0).astype(np.float32)
    return out, res


def kernel(**inputs) -> np.ndarray:
    out, _ = run(inputs, trace=False)
    return out
